# revision 1
# baseline (speedup 1.0000x reference)
"""MetalSite GNN kernel for 8-NeuronCore trn2 (axon).

Strategy: data-parallel over the batch dim (B=4 independent protein graphs),
one graph per core via jax.pmap on 4 NeuronCores; weights replicated.
The whole network (kNN build, edge features, 4 attention layers, heads) is
fused into a single jitted program per core, so all intermediates stay
on-device and the [N,N] distance work is done once per graph.
"""

import numpy as np
import jax
import jax.numpy as jnp

B, N, C_NODE, C_EDGE, H, TOP_K, N_LAYERS = 4, 2000, 1024, 128, 128, 30, 4
NUM_RBF = 16
NUM_PE = 16
NUM_HEADS = 4
FF = 4 * H
LN_EPS = 1e-5


def _ln(x, g, b):
    m = x.mean(-1, keepdims=True)
    v = ((x - m) ** 2).mean(-1, keepdims=True)
    return (x - m) / jnp.sqrt(v + LN_EPS) * g + b


def _lin(x, W, b=None):
    y = x @ W.T
    return y if b is None else y + b


def _gather(nodes, idx):
    # nodes [B,N,C], idx [B,N,K] -> [B,N,K,C]
    return jax.vmap(lambda nb, ib: nb[ib])(nodes, idx)


def _edge_features(X, mask, params):
    Bn, Nn = mask.shape
    mask2d = mask[:, :, None] * mask[:, None, :]
    dX = X[:, :, None, :] - X[:, None, :, :]
    D = mask2d * jnp.sqrt((dX**2).sum(-1) + 1e-6)
    D_max = D.max(-1, keepdims=True)
    D_adj = D + (1.0 - mask2d) * D_max
    negD, E_idx = jax.lax.top_k(-D_adj, TOP_K)
    D_nb = -negD
    D_mu = jnp.linspace(2.0, 22.0, NUM_RBF)
    D_sig = (22.0 - 2.0) / NUM_RBF
    RBF = jnp.exp(-(((D_nb[..., None] - D_mu) / D_sig) ** 2))
    ii = jnp.arange(Nn)[None, :, None]
    dpos = (E_idx - ii).astype(jnp.float32)[..., None]
    freq = jnp.exp(
        jnp.arange(0, NUM_PE, 2, dtype=jnp.float32) * (-np.log(10000.0) / NUM_PE)
    )
    ang = dpos * freq
    E = jnp.concatenate([jnp.cos(ang), jnp.sin(ang), RBF], -1)
    E = _ln(_lin(E, params["W_edge"]), params["ln_e_g"], params["ln_e_b"])
    return E, E_idx


def _attn(h_V, h_EV, mask_attend, p):
    Bn, Nn, Kn, _ = h_EV.shape
    d = H // NUM_HEADS
    Q = _lin(h_V, p["W_Q"]).reshape(Bn, Nn, NUM_HEADS, d)
    Kt = _lin(h_EV, p["W_K"]).reshape(Bn, Nn, Kn, NUM_HEADS, d)
    Vt = _lin(h_EV, p["W_V"]).reshape(Bn, Nn, Kn, NUM_HEADS, d)
    logits = jnp.einsum("bnhd,bnkhd->bnhk", Q, Kt) / np.sqrt(d).astype(np.float32)
    m = mask_attend[:, :, None, :]
    logits = jnp.where(m > 0, logits, jnp.finfo(jnp.float32).min)
    attend = jax.nn.softmax(logits, -1) * m
    out = jnp.einsum("bnhk,bnkhd->bnhd", attend, Vt).reshape(Bn, Nn, H)
    return _lin(out, p["W_O"])


def _forward(X, V, mask, params):
    E, E_idx = _edge_features(X, mask, params)
    h_V = _lin(V, params["W_v"], params["b_v"])
    h_E = _lin(E, params["W_e"], params["b_e"])
    mask_attend = mask[:, :, None] * _gather(mask[:, :, None], E_idx)[..., 0]
    for p in params["layers"]:
        h_EV = jnp.concatenate([h_E, _gather(h_V, E_idx)], -1)
        dh = _attn(h_V, h_EV, mask_attend, p)
        h_V = _ln(h_V + dh, p["ln1_g"], p["ln1_b"])
        dh = _lin(
            jax.nn.relu(_lin(h_V, p["W_fi"], p["b_fi"])), p["W_fo"], p["b_fo"]
        )
        h_V = _ln(h_V + dh, p["ln2_g"], p["ln2_b"])
        h_V = mask[..., None] * h_V
    heads = []
    for m in ["ZN", "CA", "MG", "MN"]:
        h = jax.nn.elu(_lin(h_V, params["W_%s1" % m], params["b_%s1" % m]))
        heads.append(_lin(h, params["W_%s2" % m], params["b_%s2" % m])[..., 0])
    return jnp.concatenate(heads, 1)


_PMAP_FN = None
_JIT_FN = None


def _get_pmap_fn(ndev):
    global _PMAP_FN
    if _PMAP_FN is None:
        _PMAP_FN = jax.pmap(
            _forward,
            in_axes=(0, 0, 0, None),
            devices=jax.devices()[:ndev],
        )
    return _PMAP_FN


def _get_jit_fn():
    global _JIT_FN
    if _JIT_FN is None:
        _JIT_FN = jax.jit(_forward)
    return _JIT_FN


def kernel(X, V, mask, params):
    X = np.asarray(X)
    V = np.asarray(V)
    mask = np.asarray(mask)
    ndev = min(len(jax.devices()), B)
    try:
        # one graph per core: [B,N,...] -> [B,1,N,...]
        fn = _get_pmap_fn(ndev)
        out = fn(X[:, None], V[:, None], mask[:, None], params)
        out = np.asarray(out).reshape(B, 4 * N)
    except Exception:
        out = np.asarray(_get_jit_fn()(X, V, mask, params))
    return out.astype(np.float32)
